# revision 14
# baseline (speedup 1.0000x reference)
"""MinGRU cell kernel for Trainium2, 8 NeuronCores, data-parallel over batch.

Reference computation (per batch b):
    z = x @ Wz.T + bz ; g = sigmoid(z)          [T, H]
    u = x @ Wh.T + bh                            [T, H]
    h_t = (1 - g_t) * h_{t-1} + g_t * u_t        scan over T
Output hs [B, T, H].

Per-core plan (core b handles batch b, B == 8 == n_cores), PATH="bf16_dma":
  - GPSIMD (SWDGE) cast-DMA loads x f32 HBM -> bf16 SBUF [t, d]
  - HWDGE xbar DMA-transpose (2-byte) flips 128x128 blocks -> xT [d, t]
  - PE does ONLY the z/u matmuls (bf16 in, fp32 PSUM out)
  - a = sigmoid(-z - bz) = 1 - g  (ScalarE, fused scale/bias, f32)
    g = 1 - a                     (GPSIMD tensor_scalar)
    b = (u + bh) * g              (VectorE scalar_tensor_tensor)
  - h = a * h_prev + b: hardware linear-recurrence scan along free dim
    (VectorE tensor_tensor_scan, fp32 state, chunk-chained carry)
  - hs f32 -> bf16 (VectorE copy), xbar DMA-transpose -> [t, h],
    GPSIMD cast-DMA upcasts bf16 -> f32 on the HBM store
  PATH="f32r_pe" is the exact-layout fallback: PE transposes f32 blocks
  through PSUM and matmuls run in f32r.
"""

import sys

sys.path.insert(0, "/opt/trn_rl_repo")

from contextlib import ExitStack

import numpy as np

import bass_rust
import concourse.bass as bass
import concourse.mybir as mybir
import concourse.tile as tile
from concourse.bass_utils import run_bass_kernel_spmd
from concourse.masks import make_identity

B, T, D, H = 8, 4096, 256, 256
P = 128
TC = 512          # t-chunk (PSUM bank = 512 fp32)
NCH = T // TC     # 8 chunks
NB = TC // P      # 4 t-blocks per chunk
F32 = mybir.dt.float32
F32R = mybir.dt.float32r
BF16 = mybir.dt.bfloat16
AOP = mybir.AluOpType

# knobs
PATH = "bf16_dma"   # "bf16_dma" | "f32r_pe"
N_CORES = 8


def _split_sync_waits(nc, max_waits=1):
    """walrus CoreV3 here accepts at most 1 sync-wait command per
    instruction; move excess waits onto preceding same-engine NoOps."""
    n = 0
    cnt = [0]
    for f in nc.m.functions:
        for bb in f.blocks:
            out = []
            changed = False
            for inst in bb.instructions:
                si = inst.sync_info
                if si is not None and si.on_wait and len(si.on_wait) > max_waits:
                    waits = list(si.on_wait)
                    extra, keep = waits[:-max_waits], waits[-max_waits:]
                    for j in range(0, len(extra), max_waits):
                        cnt[0] += 1
                        nop = bass_rust.InstNoOp(
                            name=f"I-waitsplit-{cnt[0]}", engine=inst.engine
                        )
                        nop.sync_info = mybir.SyncInfo(
                            on_wait=extra[j : j + max_waits], on_update=[]
                        )
                        out.append(nop)
                    inst.sync_info = mybir.SyncInfo(
                        on_wait=keep, on_update=list(si.on_update or [])
                    )
                    changed = True
                    n += 1
                out.append(inst)
            if changed:
                bb.instructions = out
    return n


def build_nc(reps=1):
    nc = bass.Bass()
    x = nc.dram_tensor("x", [T, D], F32, kind="ExternalInput")
    h0 = nc.dram_tensor("h0", [H], F32, kind="ExternalInput")
    Wz = nc.dram_tensor("Wz", [H, D], F32, kind="ExternalInput")
    bz = nc.dram_tensor("bz", [H], F32, kind="ExternalInput")
    Wh = nc.dram_tensor("Wh", [H, D], F32, kind="ExternalInput")
    bh = nc.dram_tensor("bh", [H], F32, kind="ExternalInput")
    out = nc.dram_tensor("out", [T, H], F32, kind="ExternalOutput")
    tens = (x, h0, Wz, bz, Wh, bh, out)

    with tile.TileContext(nc) as tc, ExitStack() as ctx:
        pools = {
            "consts": ctx.enter_context(tc.tile_pool(name="consts", bufs=1)),
            "xnat": ctx.enter_context(tc.tile_pool(name="xnat", bufs=3)),
            "xt": ctx.enter_context(tc.tile_pool(name="xt", bufs=4)),
            "gates": ctx.enter_context(tc.tile_pool(name="gates", bufs=3)),
            "hs": ctx.enter_context(tc.tile_pool(name="hs", bufs=4)),
            "hsb": ctx.enter_context(tc.tile_pool(name="hsb", bufs=4)),
            "outb": ctx.enter_context(tc.tile_pool(name="outb", bufs=3)),
            "xbf": ctx.enter_context(tc.tile_pool(name="xbf", bufs=3)),
            "zu": ctx.enter_context(tc.tile_pool(name="zu", bufs=4, space="PSUM")),
        }
        if PATH == "f32r_pe":
            pools["xtps"] = ctx.enter_context(
                tc.tile_pool(name="xtps", bufs=2, space="PSUM")
            )
            pools["ops"] = ctx.enter_context(
                tc.tile_pool(name="ops", bufs=2, space="PSUM")
            )
        for _rep in range(reps):
            if PATH == "bf16_dma":
                _emit_bf16_dma(nc, pools, tens)
            else:
                _emit_f32r_pe(nc, pools, tens)

    _split_sync_waits(nc)
    return nc


def _load_cols(nc, consts, name, dram):
    """[256] f32 dram vector -> [128, 2] per-partition tile."""
    t = consts.tile([P, 2], F32, tag=name)
    nc.gpsimd.dma_start(out=t, in_=dram[:].rearrange("(n p) -> p n", p=P))
    return t


# ---------------------------------------------------------------- bf16 path

def _emit_bf16_dma(nc, pools, tens):
    x, h0, Wz, bz, Wh, bh, out = tens
    consts, xnat_p, xt_p = pools["consts"], pools["xnat"], pools["xt"]
    gates_p, hs_p, hsb_p, outb_p = (
        pools["gates"], pools["hs"], pools["hsb"], pools["outb"],
    )
    xbf_p, zu_p = pools["xbf"], pools["zu"]

    bz_sb = _load_cols(nc, consts, "bz_sb", bz)
    bh_sb = _load_cols(nc, consts, "bh_sb", bh)
    h0_sb = _load_cols(nc, consts, "h0_sb", h0)
    nbz_sb = consts.tile([P, 2], F32)
    nc.vector.tensor_scalar_mul(nbz_sb, bz_sb, -1.0)

    # weights: f32 load, ACT cast -> bf16, ONE xbar transpose each.
    # WT3 [128(d_low), 4(e = hh*2+kk), 128(h_low)]; lhsT slice = WT3[:, hh*2+kk, :]
    def load_wt(name, dram):
        nat = xnat_p.tile([P, 2, D], F32, tag="x_nat")
        nc.gpsimd.dma_start(
            out=nat, in_=dram[:, :].rearrange("(n p) d -> p n d", p=P)
        )
        natb = xbf_p.tile([P, 2, D], BF16, tag="x_bf16")
        nc.scalar.copy(natb, nat)
        wt3 = consts.tile([P, 4, P], BF16, tag=f"{name}3")
        nc.sync.dma_start(
            out=wt3, in_=natb.rearrange("p a d -> p (a d)"), transpose=True
        )
        return wt3

    WzT3 = load_wt("WzT", Wz)
    WhT3 = load_wt("WhT", Wh)

    hs_prev = [None, None]
    for c in range(NCH):
        t0 = c * TC
        # 1) f32 load on SWDGE (keeps HWDGE queues pure-transpose: the xbar
        #    serializes on DMACopy <-> DMATranspose mode transitions)
        xf = xnat_p.tile([P, NB, D], F32, tag="x_nat")
        nc.gpsimd.dma_start(
            out=xf, in_=x[t0 : t0 + TC, :].rearrange("(n p) d -> p n d", p=P)
        )
        # 2) ACT cast f32 -> bf16, reordering so kk is outermost:
        #    xb [128(t), 2(kk), 4(n), 128(dd)]
        xb = xbf_p.tile([P, 2, NB, P], BF16, tag="x_bf16")
        nc.scalar.copy(
            xb.rearrange("p k n dd -> p n k dd"),
            xf.rearrange("p n (k dd) -> p n k dd", k=2),
        )
        # 3) ONE xbar transpose: [128(t), 1024] -> [128(d), 8(e=kk*4+n), 128(t)]
        xT3 = xt_p.tile([P, 2 * NB, P], BF16, tag="xT3")
        nc.sync.dma_start(
            out=xT3, in_=xb.rearrange("p k n dd -> p (k n dd)"), transpose=True
        )
        # 4) matmuls + gates + scan per h-half
        hs_cur = [None, None]
        for hh in range(2):
            z_ps = zu_p.tile([P, TC], F32, tag="zu_ps")
            u_ps = zu_p.tile([P, TC], F32, tag="zu_ps")
            for kk in range(2):
                nc.tensor.matmul(
                    z_ps, WzT3[:, hh * 2 + kk, :], xT3[:, kk * NB : (kk + 1) * NB, :],
                    start=(kk == 0), stop=(kk == 1),
                )
            for kk in range(2):
                nc.tensor.matmul(
                    u_ps, WhT3[:, hh * 2 + kk, :], xT3[:, kk * NB : (kk + 1) * NB, :],
                    start=(kk == 0), stop=(kk == 1),
                )
            a_sb = gates_p.tile([P, TC], F32, tag="a_sb")
            nc.scalar.activation(
                a_sb, z_ps, mybir.ActivationFunctionType.Sigmoid,
                bias=nbz_sb[:, hh : hh + 1], scale=-1.0,
            )
            g_sb = gates_p.tile([P, TC], F32, tag="g_sb")
            nc.gpsimd.tensor_scalar(g_sb, a_sb, -1.0, 1.0, AOP.mult, AOP.add)
            b_sb = gates_p.tile([P, TC], F32, tag="b_sb")
            nc.vector.scalar_tensor_tensor(
                b_sb, u_ps, bh_sb[:, hh : hh + 1], g_sb, AOP.add, AOP.mult
            )
            hs = hs_p.tile([P, TC], F32, tag="hs")
            init = (
                h0_sb[:, hh : hh + 1] if c == 0 else hs_prev[hh][:, TC - 1 : TC]
            )
            nc.vector.tensor_tensor_scan(hs, a_sb, b_sb, init, AOP.mult, AOP.add)
            hs_cur[hh] = hs
        hs_prev = hs_cur
        # 5) cast hs -> bf16, ONE xbar transpose per hh, upcast SWDGE store
        out_nat = outb_p.tile([P, NB, H], BF16, tag="out_sb")
        for hh in range(2):
            hsb = hsb_p.tile([P, TC], BF16, tag="hs_bf16")
            nc.vector.tensor_copy(hsb, hs_cur[hh])
            nc.sync.dma_start(
                out=out_nat[:, :, hh * P : (hh + 1) * P], in_=hsb, transpose=True
            )
        nc.gpsimd.dma_start(
            out=out[t0 : t0 + TC, :].rearrange("(n p) h -> p n h", p=P),
            in_=out_nat,
        )


# ---------------------------------------------------------------- f32r path

def _emit_f32r_pe(nc, pools, tens):
    x, h0, Wz, bz, Wh, bh, out = tens
    consts, xnat_p, xt_p = pools["consts"], pools["xnat"], pools["xt"]
    gates_p, hs_p, outb_p = pools["gates"], pools["hs"], pools["outb"]
    xtps_p, zu_p, ops_p = pools["xtps"], pools["zu"], pools["ops"]

    ident = consts.tile([P, P], F32)
    make_identity(nc, ident)

    bz_sb = _load_cols(nc, consts, "bz_sb", bz)
    bh_sb = _load_cols(nc, consts, "bh_sb", bh)
    h0_sb = _load_cols(nc, consts, "h0_sb", h0)
    nbz_sb = consts.tile([P, 2], F32)
    nc.vector.tensor_scalar_mul(nbz_sb, bz_sb, -1.0)

    def load_wt(name, dram):
        nat = xnat_p.tile([P, 2, D], F32, tag="x_nat")
        nc.sync.dma_start(
            out=nat, in_=dram[:, :].rearrange("(n p) d -> p n d", p=P)
        )
        tiles = []
        for kk in range(2):
            ps = xtps_p.tile([P, TC], F32, tag="xT_ps")
            for hh in range(2):
                nc.tensor.transpose(
                    ps[:, hh * P : (hh + 1) * P],
                    nat[:, hh, kk * P : (kk + 1) * P],
                    ident,
                )
            sb = consts.tile([P, H], F32R, tag=f"{name}_{kk}")
            nc.scalar.copy(sb, ps[:, 0:H])
            tiles.append(sb)
        return tiles

    WzT = load_wt("WzT", Wz)
    WhT = load_wt("WhT", Wh)

    hs_prev = [None, None]
    for c in range(NCH):
        t0 = c * TC
        x_nat = xnat_p.tile([P, NB, D], F32, tag="x_nat")
        nc.sync.dma_start(
            out=x_nat, in_=x[t0 : t0 + TC, :].rearrange("(n p) d -> p n d", p=P)
        )
        xT = []
        for kk in range(2):
            ps = xtps_p.tile([P, TC], F32, tag="xT_ps")
            for n in range(NB):
                nc.tensor.transpose(
                    ps[:, n * P : (n + 1) * P],
                    x_nat[:, n, kk * P : (kk + 1) * P],
                    ident,
                )
            sb = xt_p.tile([P, TC], F32R, tag="xT_sb")
            nc.scalar.copy(sb, ps)
            xT.append(sb)
        hs_cur = [None, None]
        for hh in range(2):
            z_ps = zu_p.tile([P, TC], F32, tag="zu_ps")
            u_ps = zu_p.tile([P, TC], F32, tag="zu_ps")
            for kk in range(2):
                nc.tensor.matmul(
                    z_ps, WzT[kk][:, hh * P : (hh + 1) * P], xT[kk],
                    start=(kk == 0), stop=(kk == 1),
                )
            for kk in range(2):
                nc.tensor.matmul(
                    u_ps, WhT[kk][:, hh * P : (hh + 1) * P], xT[kk],
                    start=(kk == 0), stop=(kk == 1),
                )
            a_sb = gates_p.tile([P, TC], F32, tag="a_sb")
            nc.scalar.activation(
                a_sb, z_ps, mybir.ActivationFunctionType.Sigmoid,
                bias=nbz_sb[:, hh : hh + 1], scale=-1.0,
            )
            g_sb = gates_p.tile([P, TC], F32, tag="g_sb")
            nc.gpsimd.tensor_scalar(g_sb, a_sb, -1.0, 1.0, AOP.mult, AOP.add)
            b_sb = gates_p.tile([P, TC], F32, tag="b_sb")
            nc.vector.scalar_tensor_tensor(
                b_sb, u_ps, bh_sb[:, hh : hh + 1], g_sb, AOP.add, AOP.mult
            )
            hs = hs_p.tile([P, TC], F32, tag="hs")
            init = (
                h0_sb[:, hh : hh + 1] if c == 0 else hs_prev[hh][:, TC - 1 : TC]
            )
            nc.vector.tensor_tensor_scan(hs, a_sb, b_sb, init, AOP.mult, AOP.add)
            hs_cur[hh] = hs
        hs_prev = hs_cur
        out_sb = outb_p.tile([P, NB * H], F32, tag="out_sb")
        for q in range(2):
            ps = ops_p.tile([P, TC], F32, tag="out_ps")
            for dn in range(2):
                for hh in range(2):
                    n = 2 * q + dn
                    nc.tensor.transpose(
                        ps[:, dn * H + hh * P : dn * H + (hh + 1) * P],
                        hs_cur[hh][:, n * P : (n + 1) * P],
                        ident,
                    )
            if q == 0:
                nc.scalar.copy(out_sb[:, q * TC : (q + 1) * TC], ps)
            else:
                nc.vector.tensor_copy(out_sb[:, q * TC : (q + 1) * TC], ps)
        nc.sync.dma_start(
            out=out[t0 : t0 + TC, :].rearrange("(n p) h -> p n h", p=P),
            in_=out_sb.rearrange("p (n h) -> p n h", n=NB),
        )


_NC_CACHE = {}


def _get_nc(reps=1):
    if reps not in _NC_CACHE:
        _NC_CACHE[reps] = build_nc(reps)
    return _NC_CACHE[reps]


def kernel(x, h0, Wz, bz, Wh, bh):
    x = np.ascontiguousarray(np.asarray(x, dtype=np.float32))
    h0 = np.ascontiguousarray(np.asarray(h0, dtype=np.float32))
    Wz = np.ascontiguousarray(np.asarray(Wz, dtype=np.float32))
    bz = np.ascontiguousarray(np.asarray(bz, dtype=np.float32))
    Wh = np.ascontiguousarray(np.asarray(Wh, dtype=np.float32))
    bh = np.ascontiguousarray(np.asarray(bh, dtype=np.float32))
    nc = _get_nc(1)
    in_maps = [
        {"x": x[b], "h0": h0[b], "Wz": Wz, "bz": bz, "Wh": Wh, "bh": bh}
        for b in range(N_CORES)
    ]
    res = run_bass_kernel_spmd(nc, in_maps, list(range(N_CORES))).results
    return np.stack([res[b]["out"] for b in range(N_CORES)], axis=0)


# revision 15
# speedup vs baseline: 13.6052x; 13.6052x over previous
"""MinGRU cell kernel for Trainium2, 8 NeuronCores, data-parallel over batch.

Reference computation (per batch b):
    z = x @ Wz.T + bz ; g = sigmoid(z)          [T, H]
    u = x @ Wh.T + bh                            [T, H]
    h_t = (1 - g_t) * h_{t-1} + g_t * u_t        scan over T
Output hs [B, T, H].

Per-core plan (core b handles batch b, B == 8 == n_cores), PATH="bf16_dma":
  - GPSIMD (SWDGE) cast-DMA loads x f32 HBM -> bf16 SBUF [t, d]
  - HWDGE xbar DMA-transpose (2-byte) flips 128x128 blocks -> xT [d, t]
  - PE does ONLY the z/u matmuls (bf16 in, fp32 PSUM out)
  - a = sigmoid(-z - bz) = 1 - g  (ScalarE, fused scale/bias, f32)
    g = 1 - a                     (GPSIMD tensor_scalar)
    b = (u + bh) * g              (VectorE scalar_tensor_tensor)
  - h = a * h_prev + b: hardware linear-recurrence scan along free dim
    (VectorE tensor_tensor_scan, fp32 state, chunk-chained carry)
  - hs f32 -> bf16 (VectorE copy), xbar DMA-transpose -> [t, h],
    GPSIMD cast-DMA upcasts bf16 -> f32 on the HBM store
  PATH="f32r_pe" is the exact-layout fallback: PE transposes f32 blocks
  through PSUM and matmuls run in f32r.
"""

import sys

sys.path.insert(0, "/opt/trn_rl_repo")

from contextlib import ExitStack

import numpy as np

import bass_rust
import concourse.bass as bass
import concourse.mybir as mybir
import concourse.tile as tile
from concourse.bass_utils import run_bass_kernel_spmd
from concourse.masks import make_identity

B, T, D, H = 8, 4096, 256, 256
P = 128
TC = 512          # t-chunk (PSUM bank = 512 fp32)
NCH = T // TC     # 8 chunks
NB = TC // P      # 4 t-blocks per chunk
F32 = mybir.dt.float32
F32R = mybir.dt.float32r
BF16 = mybir.dt.bfloat16
AOP = mybir.AluOpType

# knobs
PATH = "bf16_dma"   # "bf16_dma" | "f32r_pe"
N_CORES = 8


def _split_sync_waits(nc, max_waits=1):
    """walrus CoreV3 here accepts at most 1 sync-wait command per
    instruction; move excess waits onto preceding same-engine NoOps."""
    n = 0
    cnt = [0]
    for f in nc.m.functions:
        for bb in f.blocks:
            out = []
            changed = False
            for inst in bb.instructions:
                si = inst.sync_info
                if si is not None and si.on_wait and len(si.on_wait) > max_waits:
                    waits = list(si.on_wait)
                    extra, keep = waits[:-max_waits], waits[-max_waits:]
                    for j in range(0, len(extra), max_waits):
                        cnt[0] += 1
                        nop = bass_rust.InstNoOp(
                            name=f"I-waitsplit-{cnt[0]}", engine=inst.engine
                        )
                        nop.sync_info = mybir.SyncInfo(
                            on_wait=extra[j : j + max_waits], on_update=[]
                        )
                        out.append(nop)
                    inst.sync_info = mybir.SyncInfo(
                        on_wait=keep, on_update=list(si.on_update or [])
                    )
                    changed = True
                    n += 1
                out.append(inst)
            if changed:
                bb.instructions = out
    return n


def build_nc(reps=1):
    nc = bass.Bass()
    x = nc.dram_tensor("x", [T, D], F32, kind="ExternalInput")
    h0 = nc.dram_tensor("h0", [H], F32, kind="ExternalInput")
    Wz = nc.dram_tensor("Wz", [H, D], F32, kind="ExternalInput")
    bz = nc.dram_tensor("bz", [H], F32, kind="ExternalInput")
    Wh = nc.dram_tensor("Wh", [H, D], F32, kind="ExternalInput")
    bh = nc.dram_tensor("bh", [H], F32, kind="ExternalInput")
    out = nc.dram_tensor("out", [T, H], F32, kind="ExternalOutput")
    tens = (x, h0, Wz, bz, Wh, bh, out)

    with tile.TileContext(nc) as tc, ExitStack() as ctx:
        pools = {
            "consts": ctx.enter_context(tc.tile_pool(name="consts", bufs=1)),
            "xnat": ctx.enter_context(tc.tile_pool(name="xnat", bufs=3)),
            "xt": ctx.enter_context(tc.tile_pool(name="xt", bufs=4)),
            "gates": ctx.enter_context(tc.tile_pool(name="gates", bufs=3)),
            "hs": ctx.enter_context(tc.tile_pool(name="hs", bufs=4)),
            "hsb": ctx.enter_context(tc.tile_pool(name="hsb", bufs=4)),
            "outb": ctx.enter_context(tc.tile_pool(name="outb", bufs=3)),
            "xbf": ctx.enter_context(tc.tile_pool(name="xbf", bufs=3)),
            "zu": ctx.enter_context(tc.tile_pool(name="zu", bufs=4, space="PSUM")),
        }
        if PATH == "f32r_pe":
            pools["xtps"] = ctx.enter_context(
                tc.tile_pool(name="xtps", bufs=2, space="PSUM")
            )
            pools["ops"] = ctx.enter_context(
                tc.tile_pool(name="ops", bufs=2, space="PSUM")
            )
        for _rep in range(reps):
            if PATH == "bf16_dma":
                _emit_bf16_dma(nc, pools, tens)
            else:
                _emit_f32r_pe(nc, pools, tens)

    _split_sync_waits(nc)
    return nc


def _load_cols(nc, consts, name, dram):
    """[256] f32 dram vector -> [128, 2] per-partition tile."""
    t = consts.tile([P, 2], F32, tag=name)
    nc.gpsimd.dma_start(out=t, in_=dram[:].rearrange("(n p) -> p n", p=P))
    return t


# ---------------------------------------------------------------- bf16 path

def _emit_bf16_dma(nc, pools, tens):
    x, h0, Wz, bz, Wh, bh, out = tens
    consts, xnat_p, xt_p = pools["consts"], pools["xnat"], pools["xt"]
    gates_p, hs_p, hsb_p, outb_p = (
        pools["gates"], pools["hs"], pools["hsb"], pools["outb"],
    )
    xbf_p, zu_p = pools["xbf"], pools["zu"]

    bz_sb = _load_cols(nc, consts, "bz_sb", bz)
    bh_sb = _load_cols(nc, consts, "bh_sb", bh)
    h0_sb = _load_cols(nc, consts, "h0_sb", h0)
    nbz_sb = consts.tile([P, 2], F32)
    nc.vector.tensor_scalar_mul(nbz_sb, bz_sb, -1.0)

    # weights: HWDGE f32 load, DVE cast -> bf16, ONE xbar transpose each.
    # WT3 [128(d_low), 4(e = hh*2+kk), 128(h_low)]; lhsT slice = WT3[:, hh*2+kk, :]
    def load_wt(name, dram):
        nat = xnat_p.tile([P, 2, D], F32, tag="x_nat")
        nc.sync.dma_start(
            out=nat, in_=dram[:, :].rearrange("(n p) d -> p n d", p=P)
        )
        natb = xbf_p.tile([P, 2, D], BF16, tag="x_bf16")
        nc.vector.tensor_copy(natb, nat)
        wt3 = consts.tile([P, 4, P], BF16, tag=f"{name}3")
        nc.scalar.dma_start(
            out=wt3, in_=natb.rearrange("p a d -> p (a d)"), transpose=True
        )
        return wt3

    WzT3 = load_wt("WzT", Wz)
    WhT3 = load_wt("WhT", Wh)

    hs_prev = [None, None]
    for c in range(NCH):
        t0 = c * TC
        # 1) f32 load on SP HWDGE (plain copies); all xbar transposes go on
        #    the Activation HWDGE engine so copy<->transpose xbar-mode
        #    transitions never alternate within one issue engine
        xf = xnat_p.tile([P, NB, D], F32, tag="x_nat")
        nc.sync.dma_start(
            out=xf, in_=x[t0 : t0 + TC, :].rearrange("(n p) d -> p n d", p=P)
        )
        # 2) DVE cast f32 -> bf16, reordering so kk is outermost:
        #    xb [128(t), 2(kk), 4(n), 128(dd)]
        xb = xbf_p.tile([P, 2, NB, P], BF16, tag="x_bf16")
        nc.vector.tensor_copy(
            xb.rearrange("p k n dd -> p n k dd"),
            xf.rearrange("p n (k dd) -> p n k dd", k=2),
        )
        # 3) ONE xbar transpose: [128(t), 1024] -> [128(d), 8(e=kk*4+n), 128(t)]
        xT3 = xt_p.tile([P, 2 * NB, P], BF16, tag="xT3")
        nc.scalar.dma_start(
            out=xT3, in_=xb.rearrange("p k n dd -> p (k n dd)"), transpose=True
        )
        # 4) matmuls + gates + scan per h-half
        hs_cur = [None, None]
        for hh in range(2):
            z_ps = zu_p.tile([P, TC], F32, tag="zu_ps")
            u_ps = zu_p.tile([P, TC], F32, tag="zu_ps")
            for kk in range(2):
                nc.tensor.matmul(
                    z_ps, WzT3[:, hh * 2 + kk, :], xT3[:, kk * NB : (kk + 1) * NB, :],
                    start=(kk == 0), stop=(kk == 1),
                )
            for kk in range(2):
                nc.tensor.matmul(
                    u_ps, WhT3[:, hh * 2 + kk, :], xT3[:, kk * NB : (kk + 1) * NB, :],
                    start=(kk == 0), stop=(kk == 1),
                )
            # a = 1 - g = sigmoid(-z - bz)
            a_sb = gates_p.tile([P, TC], F32, tag="a_sb")
            nc.scalar.activation(
                a_sb, z_ps, mybir.ActivationFunctionType.Sigmoid,
                bias=nbz_sb[:, hh : hh + 1], scale=-1.0,
            )
            # v = u + bh ; nb = (a - 1) * v = -g*v ; h = a*h - nb
            v_sb = gates_p.tile([P, TC], F32, tag="v_sb")
            nc.vector.tensor_scalar_add(v_sb, u_ps, bh_sb[:, hh : hh + 1])
            nb_sb = gates_p.tile([P, TC], F32, tag="nb_sb")
            nc.vector.scalar_tensor_tensor(
                nb_sb, a_sb, 1.0, v_sb, AOP.subtract, AOP.mult
            )
            hs = hs_p.tile([P, TC], F32, tag="hs")
            init = (
                h0_sb[:, hh : hh + 1] if c == 0 else hs_prev[hh][:, TC - 1 : TC]
            )
            nc.vector.tensor_tensor_scan(
                hs, a_sb, nb_sb, init, AOP.mult, AOP.subtract
            )
            hs_cur[hh] = hs
        hs_prev = hs_cur
        # 5) DVE cast hs -> bf16, ONE xbar transpose per hh,
        #    GPSIMD upcast copy, HWDGE f32 store
        out_nat = outb_p.tile([P, NB, H], BF16, tag="out_sb")
        for hh in range(2):
            hsb = hsb_p.tile([P, TC], BF16, tag="hs_bf16")
            nc.vector.tensor_copy(hsb, hs_cur[hh])
            nc.scalar.dma_start(
                out=out_nat[:, :, hh * P : (hh + 1) * P], in_=hsb, transpose=True
            )
        out_f32 = outb_p.tile([P, NB, H], F32, tag="out_f32")
        nc.gpsimd.tensor_copy(out_f32, out_nat)
        nc.sync.dma_start(
            out=out[t0 : t0 + TC, :].rearrange("(n p) h -> p n h", p=P),
            in_=out_f32,
        )


# ---------------------------------------------------------------- f32r path

def _emit_f32r_pe(nc, pools, tens):
    x, h0, Wz, bz, Wh, bh, out = tens
    consts, xnat_p, xt_p = pools["consts"], pools["xnat"], pools["xt"]
    gates_p, hs_p, outb_p = pools["gates"], pools["hs"], pools["outb"]
    xtps_p, zu_p, ops_p = pools["xtps"], pools["zu"], pools["ops"]

    ident = consts.tile([P, P], F32)
    make_identity(nc, ident)

    bz_sb = _load_cols(nc, consts, "bz_sb", bz)
    bh_sb = _load_cols(nc, consts, "bh_sb", bh)
    h0_sb = _load_cols(nc, consts, "h0_sb", h0)
    nbz_sb = consts.tile([P, 2], F32)
    nc.vector.tensor_scalar_mul(nbz_sb, bz_sb, -1.0)

    def load_wt(name, dram):
        nat = xnat_p.tile([P, 2, D], F32, tag="x_nat")
        nc.sync.dma_start(
            out=nat, in_=dram[:, :].rearrange("(n p) d -> p n d", p=P)
        )
        tiles = []
        for kk in range(2):
            ps = xtps_p.tile([P, TC], F32, tag="xT_ps")
            for hh in range(2):
                nc.tensor.transpose(
                    ps[:, hh * P : (hh + 1) * P],
                    nat[:, hh, kk * P : (kk + 1) * P],
                    ident,
                )
            sb = consts.tile([P, H], F32R, tag=f"{name}_{kk}")
            nc.scalar.copy(sb, ps[:, 0:H])
            tiles.append(sb)
        return tiles

    WzT = load_wt("WzT", Wz)
    WhT = load_wt("WhT", Wh)

    hs_prev = [None, None]
    for c in range(NCH):
        t0 = c * TC
        x_nat = xnat_p.tile([P, NB, D], F32, tag="x_nat")
        nc.sync.dma_start(
            out=x_nat, in_=x[t0 : t0 + TC, :].rearrange("(n p) d -> p n d", p=P)
        )
        xT = []
        for kk in range(2):
            ps = xtps_p.tile([P, TC], F32, tag="xT_ps")
            for n in range(NB):
                nc.tensor.transpose(
                    ps[:, n * P : (n + 1) * P],
                    x_nat[:, n, kk * P : (kk + 1) * P],
                    ident,
                )
            sb = xt_p.tile([P, TC], F32R, tag="xT_sb")
            nc.scalar.copy(sb, ps)
            xT.append(sb)
        hs_cur = [None, None]
        for hh in range(2):
            z_ps = zu_p.tile([P, TC], F32, tag="zu_ps")
            u_ps = zu_p.tile([P, TC], F32, tag="zu_ps")
            for kk in range(2):
                nc.tensor.matmul(
                    z_ps, WzT[kk][:, hh * P : (hh + 1) * P], xT[kk],
                    start=(kk == 0), stop=(kk == 1),
                )
            for kk in range(2):
                nc.tensor.matmul(
                    u_ps, WhT[kk][:, hh * P : (hh + 1) * P], xT[kk],
                    start=(kk == 0), stop=(kk == 1),
                )
            a_sb = gates_p.tile([P, TC], F32, tag="a_sb")
            nc.scalar.activation(
                a_sb, z_ps, mybir.ActivationFunctionType.Sigmoid,
                bias=nbz_sb[:, hh : hh + 1], scale=-1.0,
            )
            g_sb = gates_p.tile([P, TC], F32, tag="g_sb")
            nc.gpsimd.tensor_scalar(g_sb, a_sb, -1.0, 1.0, AOP.mult, AOP.add)
            b_sb = gates_p.tile([P, TC], F32, tag="b_sb")
            nc.vector.scalar_tensor_tensor(
                b_sb, u_ps, bh_sb[:, hh : hh + 1], g_sb, AOP.add, AOP.mult
            )
            hs = hs_p.tile([P, TC], F32, tag="hs")
            init = (
                h0_sb[:, hh : hh + 1] if c == 0 else hs_prev[hh][:, TC - 1 : TC]
            )
            nc.vector.tensor_tensor_scan(hs, a_sb, b_sb, init, AOP.mult, AOP.add)
            hs_cur[hh] = hs
        hs_prev = hs_cur
        out_sb = outb_p.tile([P, NB * H], F32, tag="out_sb")
        for q in range(2):
            ps = ops_p.tile([P, TC], F32, tag="out_ps")
            for dn in range(2):
                for hh in range(2):
                    n = 2 * q + dn
                    nc.tensor.transpose(
                        ps[:, dn * H + hh * P : dn * H + (hh + 1) * P],
                        hs_cur[hh][:, n * P : (n + 1) * P],
                        ident,
                    )
            if q == 0:
                nc.scalar.copy(out_sb[:, q * TC : (q + 1) * TC], ps)
            else:
                nc.vector.tensor_copy(out_sb[:, q * TC : (q + 1) * TC], ps)
        nc.sync.dma_start(
            out=out[t0 : t0 + TC, :].rearrange("(n p) h -> p n h", p=P),
            in_=out_sb.rearrange("p (n h) -> p n h", n=NB),
        )


_NC_CACHE = {}


def _get_nc(reps=1):
    if reps not in _NC_CACHE:
        _NC_CACHE[reps] = build_nc(reps)
    return _NC_CACHE[reps]


def kernel(x, h0, Wz, bz, Wh, bh):
    x = np.ascontiguousarray(np.asarray(x, dtype=np.float32))
    h0 = np.ascontiguousarray(np.asarray(h0, dtype=np.float32))
    Wz = np.ascontiguousarray(np.asarray(Wz, dtype=np.float32))
    bz = np.ascontiguousarray(np.asarray(bz, dtype=np.float32))
    Wh = np.ascontiguousarray(np.asarray(Wh, dtype=np.float32))
    bh = np.ascontiguousarray(np.asarray(bh, dtype=np.float32))
    nc = _get_nc(1)
    in_maps = [
        {"x": x[b], "h0": h0[b], "Wz": Wz, "bz": bz, "Wh": Wh, "bh": bh}
        for b in range(N_CORES)
    ]
    res = run_bass_kernel_spmd(nc, in_maps, list(range(N_CORES))).results
    return np.stack([res[b]["out"] for b in range(N_CORES)], axis=0)
